# revision 5
# baseline (speedup 1.0000x reference)
"""TRN2 Bass/Tile kernel for nn_BlockSparseMoE (T=2048, D=1024, F=2048, E=8, top-2).

Grouped expert-tensor parallelism across the 8 NeuronCores: experts are
packed into groups of G (G=2 default); the G cores of a group each own an
F/G slice of ALL G experts in the group and process ALL tokens routed to
those experts. Per-core PE cycles then scale with the group's slot-capacity
SUM (~G*512 after host load balancing) instead of the per-expert MAX
(548), and every slot with cap<=512 runs single-chunk 500+-column matmuls
instead of 274-column pairs.

The host performs routing (top-2 of an [T, E] logit matmul) and the
dispatch/combine: it gathers each slot's tokens into a compact d-major
activation block, and after the device run sums the G partial outputs of a
group (bf16 [D, S] each, disjoint f-slices), scales by the combine
coefficient and scatters into the full [T, D] output.

Device NEFF (SPMD-uniform; per-core weights/tokens differ only in data):

  M12  a = W1 @ xc, b = V1 @ xc  (f-major [128f, cap_j] PSUM chains over 8
       d-tiles, 16 f-units = G slots x 16/G local f-tiles; <=512-wide
       chunks paired under each stationary 128x128 block)
       hT[u] = silu(a) * b  (ACT Silu + 1 DVE mult, bf16 out)
  M3T  yT[d][slot j] = sum_{u in slot j} W2[u, d-block]^T-chain @ hT[u]
       ([128d, cap_j] PSUM chains over 16/G f-tiles), drained to bf16 on
       alternating ACT/DVE and DMA'd out as the [D, S] f-partial.

Weight traffic stays 12.6 MB bf16/rep for any G (each core holds F/G of G
experts); activation in / partial out grow with S. DMA rides three queues,
each chosen so nothing FIFO-blocks the next rep's prefetch: SP hwdge
carries only the w1|v1 f-unit-pair stream, ACT hwdge the xcs token blocks,
Pool SWDGE the w2 quarters and the yT partial writes (y-outs on SP cost
~12 us/rep in boundary serialization; xcs on SP delays the wv stream).
Measured 87.0-89.1 us/rep official pair-timing (92.0 before the queue
moves) vs the 102 us expert-parallel baseline; bf16 PE floor for
S=1056 is 84.5 us nominal at 2.4 GHz.
"""

import os

import numpy as np

import concourse.bass as bass  # noqa: F401  (kept for parity with tooling)
import concourse.mybir as mybir
import concourse.tile as tile
from concourse import bacc
from concourse.bass_utils import run_bass_kernel_spmd

f32 = mybir.dt.float32
bf16 = mybir.dt.bfloat16
AF = mybir.ActivationFunctionType
OP = mybir.AluOpType

np_bf16 = mybir.dt.np(bf16)

_REPS = int(os.environ.get("MOE_REPS", "1"))
G = int(os.environ.get("MOE_G", "2"))  # experts per group / cores per group
_SILU = os.environ.get("MOE_SILU", "silu") == "silu"

P = 128
T = 2048
D = 1024
F = 2048
E = 8
ND = D // P   # 8 d tiles
NF = F // P   # 16 f tiles (total across a group's slices)


def _chunks(C):
    """Split [0, C) into balanced PSUM-bank-sized (<=512) column chunks."""
    n = (C + 511) // 512
    base = C // n
    rem = C - base * n
    out, off = [], 0
    for i in range(n):
        w = base + (1 if i < rem else 0)
        out.append((off, w))
        off += w
    return out


def build_moe(caps, reps=None, mode="full", yq=None, silu=None, xq=None):
    """caps: per-slot column capacities (len G, uniform across cores).

    mode: "full" | "nodma" (weights resident) | "dmaonly" (no compute).
    """
    global _REPS
    if reps is not None:
        _REPS = reps
    if yq is None:
        yq = os.environ.get("MOE_YQ", "sw")
    if silu is None:
        silu = _SILU
    if xq is None:
        xq = os.environ.get("MOE_XQ", "act")
    g = len(caps)
    f_per = NF // g            # local f-tiles per slot
    S = sum(caps)
    offs = [sum(caps[:j]) for j in range(g)]  # column offset of slot j
    # unit u -> slot, chunk list (chunk offsets are S-global)
    u_slot = [u // f_per for u in range(NF)]
    slot_chg = []
    for j in range(g):
        chs = [(offs[j] + o, w) for o, w in _chunks(caps[j])]
        # chunk groups of <=2 bound live PSUM tiles per chain
        slot_chg.append([chs[i:i + 2] for i in range(0, len(chs), 2)])

    MMB, YB = (4, 4) if mode == "mm44" else (6, 2)

    nc = bacc.Bacc("TRN2", target_bir_lowering=False, debug=False)

    # token activations packed partition-major: [p, dt, c] = xc[c, dt*128+p]
    xcs = nc.dram_tensor("xcs", [P, ND, S], bf16, kind="ExternalInput").ap()
    # w1/v1 swizzles packed in f-unit pairs: one 8KB-row DMA per 2 units
    wv1s = nc.dram_tensor("wv1s", [NF // 2, P, 4 * ND * P], bf16,
                          kind="ExternalInput").ap()
    # all w2 units packed partition-major: [p, u, d] = w2unit[u][128p-row, d]
    w2s = nc.dram_tensor("w2s", [P, NF, D], bf16, kind="ExternalInput").ap()
    yT = nc.dram_tensor("yT", [D, S], bf16, kind="ExternalOutput").ap()

    big = S > 1536  # shrink double-buffering to fit SBUF at large S
    with tile.TileContext(nc) as tc:
        with (
            tc.tile_pool(name="xct", bufs=(2 if big else 4)) as xctpool,
            tc.tile_pool(name="w12",
                         bufs=(NF // 2 if mode == "nodma" else 3)) as wpool,
            tc.tile_pool(name="w2p",
                         bufs=(1 if mode == "nodma" else 2)) as w2pool,
            tc.tile_pool(name="ht",
                         bufs=(NF + 4 if big else 2 * NF)) as htpool,
            tc.tile_pool(name="ssb", bufs=4) as spool,
            tc.tile_pool(name="ysb", bufs=(2 if big else 3)) as ypool,
            tc.tile_pool(name="psum", bufs=1, space="PSUM") as psp,
        ):
            res_w = {}

            def _load_wv(grp):
                # f-units 2grp and 2grp+1 in one DMA (SP hwdge queue)
                wv_sb = wpool.tile([P, 4 * ND * P], bf16, tag="w12",
                                   name=f"wv1_{grp}")
                nc.sync.dma_start(out=wv_sb[:], in_=wv1s[grp, :, :])
                return wv_sb

            def _load_w2_tile():
                return w2pool.tile([P, NF, D], bf16, tag="w2", name="w2")

            def _load_w2_quarter(w2_sb, q):
                # W2 in 4 spread DMAs on the SWDGE queue (Pool engine)
                nc.gpsimd.dma_start(
                    out=w2_sb[:, 4 * q:4 * q + 4, :],
                    in_=w2s[:, 4 * q:4 * q + 4, :],
                )

            def _load_w2():
                w2_sb = _load_w2_tile()
                for q in range(4):
                    _load_w2_quarter(w2_sb, q)
                return w2_sb

            if mode == "nodma":
                for grp in range(NF // 2):
                    res_w[grp] = _load_wv(grp)
                res_w["w2"] = _load_w2()

            def _emit_dmaonly():
                for grp in range(NF // 2):
                    _load_wv(grp)
                _load_w2()

            io_eng = nc.sync
            # xcs queue: "act" rides the ACT hwdge so the (S-scaled) token
            # block never delays the wv1 weight stream on the SP queue
            xc_eng = nc.scalar if xq == "act" else io_eng
            # y-out queue: "sw" rides Pool SWDGE so the partial writes never
            # FIFO-block the next rep's wv1/xcs prefetch on the SP queue
            yo_eng = nc.gpsimd if yq == "sw" else io_eng

            def _emit_body():
                # first w1/v1 pair ahead of xcs in the SP FIFO
                wv0 = res_w[0] if mode == "nodma" else _load_wv(0)
                # compact token activations, packed [p, dt, c]; two tiles
                xc_h = [None, None]
                for q in range(2):
                    xc_h[q] = xctpool.tile([P, ND // 2, S], bf16, tag="xct",
                                           name=f"xcs_{q}")
                    xc_eng.dma_start(
                        out=xc_h[q][:], in_=xcs[:, 4 * q:4 * q + 4, :]
                    )
                xc_sb = [xc_h[d // 4][:, d % 4, :] for d in range(ND)]

                # ---- M12: hT[u] = silu(W1 xc) * (V1 xc), f-major ----
                hT = [None] * NF
                w2_sb = res_w["w2"] if mode == "nodma" else _load_w2_tile()
                for u in range(NF):
                    j = u_slot[u]
                    cap = caps[j]
                    off0 = offs[j]
                    hT[u] = htpool.tile([P, cap], bf16, tag="ht",
                                        name=f"ht_{u}")
                    if mode != "nodma" and u % 4 == 2:
                        _load_w2_quarter(w2_sb, u // 4)
                    if u == 0:
                        wv_sb = wv0
                    elif u % 2 == 0:
                        wv_sb = (res_w[u // 2] if mode == "nodma"
                                 else _load_wv(u // 2))
                    half = (u % 2) * 2 * ND * P
                    w1_sb = wv_sb[:, half:half + ND * P]
                    v1_sb = wv_sb[:, half + ND * P:half + 2 * ND * P]
                    for grp in slot_chg[j]:
                        a_ps = [psp.tile([P, 512], f32, tag="mm", bufs=MMB,
                                         name=f"a_ps_{ci}")
                                for ci in range(len(grp))]
                        for d in range(ND):
                            for ci, (off, w) in enumerate(grp):
                                nc.tensor.matmul(
                                    out=a_ps[ci][:, :w],
                                    lhsT=w1_sb[:, d * P:(d + 1) * P],
                                    rhs=xc_sb[d][:, off:off + w],
                                    start=(d == 0), stop=(d == ND - 1),
                                )
                        b_ps = [psp.tile([P, 512], f32, tag="mm", bufs=MMB,
                                         name=f"b_ps_{ci}")
                                for ci in range(len(grp))]
                        for d in range(ND):
                            for ci, (off, w) in enumerate(grp):
                                nc.tensor.matmul(
                                    out=b_ps[ci][:, :w],
                                    lhsT=v1_sb[:, d * P:(d + 1) * P],
                                    rhs=xc_sb[d][:, off:off + w],
                                    start=(d == 0), stop=(d == ND - 1),
                                )
                        for ci, (off, w) in enumerate(grp):
                            s_sb = spool.tile([P, 512], f32, tag="ssb")
                            if silu:
                                # fused silu on ACT: one PSUM read, one DVE op
                                nc.scalar.activation(
                                    s_sb[:, :w], a_ps[ci][:, :w], AF.Silu)
                            else:
                                # CoreSim-compatible fallback
                                nc.scalar.activation(
                                    s_sb[:, :w], a_ps[ci][:, :w], AF.Sigmoid)
                                nc.vector.tensor_tensor(
                                    out=s_sb[:, :w], in0=s_sb[:, :w],
                                    in1=a_ps[ci][:, :w], op=OP.mult,
                                )
                            nc.vector.tensor_tensor(
                                out=hT[u][:, off - off0:off - off0 + w],
                                in0=s_sb[:, :w],
                                in1=b_ps[ci][:, :w], op=OP.mult,
                            )

                # ---- M3T: yT[d][slot] = sum_u w2[u, d-block]^T @ hT[u] ----
                ncopy = 0
                for d in range(ND):
                    y_sb = ypool.tile([P, S], bf16, tag="ysb", name=f"y_{d}")
                    for j in range(g):
                        u0 = j * f_per
                        off0 = offs[j]
                        for grp in slot_chg[j]:
                            y_ps = [psp.tile([P, 512], f32, tag="y", bufs=YB,
                                             name=f"y_ps_{ci}")
                                    for ci in range(len(grp))]
                            for fu in range(f_per):
                                u = u0 + fu
                                for ci, (off, w) in enumerate(grp):
                                    nc.tensor.matmul(
                                        out=y_ps[ci][:, :w],
                                        lhsT=w2_sb[:, u, d * P:(d + 1) * P],
                                        rhs=hT[u][:, off - off0:off - off0 + w],
                                        start=(fu == 0),
                                        stop=(fu == f_per - 1),
                                    )
                            for ci, (off, w) in enumerate(grp):
                                # alternate PSUM drains between ACT and DVE
                                # so neither engine rate-limits short chains
                                if ncopy % 2 == 0:
                                    nc.scalar.activation(
                                        y_sb[:, off:off + w],
                                        y_ps[ci][:, :w], AF.Copy
                                    )
                                else:
                                    nc.vector.tensor_scalar_mul(
                                        y_sb[:, off:off + w],
                                        y_ps[ci][:, :w], 1.0
                                    )
                                ncopy += 1
                    yo_eng.dma_start(
                        out=yT[d * P:(d + 1) * P, :], in_=y_sb[:]
                    )

            for _rep in range(_REPS):
                if mode == "dmaonly":
                    _emit_dmaonly()
                else:
                    _emit_body()

    return nc


_NC_CACHE = {}


def _get_nc(caps, reps=None, mode="full", yq=None, silu=None, xq=None):
    caps = tuple(caps)
    key = (caps, reps if reps is not None else _REPS, mode, yq, silu, xq)
    if key not in _NC_CACHE:
        nc = build_moe(caps, reps=reps, mode=mode, yq=yq, silu=silu, xq=xq)
        nc.compile()
        _NC_CACHE[key] = nc
    return _NC_CACHE[key]


def _route(x, gate_w):
    """Host top-2 routing. Returns per-expert (token idx, combine coef)."""
    logits = x.astype(np.float32) @ gate_w.astype(np.float32).T  # [T, E]
    t = np.arange(logits.shape[0])
    sel1 = np.argmax(logits, axis=1)
    l1 = logits[t, sel1]
    masked = logits.copy()
    masked[t, sel1] = -np.inf
    sel2 = np.argmax(masked, axis=1)
    l2 = logits[t, sel2]
    w1c = 1.0 / (1.0 + np.exp(l2 - l1))
    w2c = 1.0 - w1c
    idx, cf = [], []
    for e in range(E):
        m1 = sel1 == e
        m2 = sel2 == e
        ide = np.nonzero(m1 | m2)[0]
        ce = np.where(m1[ide], w1c[ide], w2c[ide]).astype(np.float32)
        idx.append(ide)
        cf.append(ce)
    return idx, cf


def _grouping(idx):
    """Pack experts into 8//G groups of G; returns groups and slot caps.

    Groups get snake-ordered experts by load so the per-slot elementwise
    max over groups (the SPMD-uniform capacity) stays near the mean.
    """
    counts = np.array([len(i) for i in idx])
    order = np.argsort(-counts, kind="stable")
    ngr = E // G
    groups = [[] for _ in range(ngr)]
    for r, e in enumerate(order):
        q = r % ngr
        if (r // ngr) % 2 == 1:
            q = ngr - 1 - q
        groups[q].append(int(e))
    # slot k = k-th heaviest within each group; cap = max over groups
    for q in range(ngr):
        groups[q].sort(key=lambda e: -counts[e])
    caps = [max(16, int(np.ceil(
        max(counts[groups[q][k]] for q in range(ngr)) / 4) * 4))
        for k in range(G)]
    return groups, caps


def _swizzle_w1(w):
    """(f_rows, D) -> [nf, 128, ND*128] with [f, p, dt*128+fc] = w[f*128+fc, dt*128+p]."""
    nf = w.shape[0] // P
    v = w.reshape(nf, P, ND, P)  # [f, fc, dt, p]
    return np.ascontiguousarray(v.transpose(0, 3, 2, 1).reshape(nf, P, ND * P))


def _build_in_maps(x, gate_w, w1, v1, w2, caps, idx):
    groups, caps2 = _grouping(idx)
    assert tuple(caps2) == tuple(caps), (caps, caps2)
    x = np.asarray(x, dtype=np.float32)
    w1 = np.asarray(w1, dtype=np.float32)
    v1 = np.asarray(v1, dtype=np.float32)
    w2 = np.asarray(w2, dtype=np.float32)
    S = sum(caps)
    f_per = NF // G
    fh = F // G  # rows per core's f-slice
    in_maps = []
    for c in range(E):
        q, h = c // G, c % G
        rows = slice(h * fh, (h + 1) * fh)
        # activations: concatenated slot blocks
        xc = np.zeros((S, D), dtype=np_bf16)
        off = 0
        for k in range(G):
            ide = idx[groups[q][k]]
            xc[off:off + len(ide)] = x[ide].astype(np_bf16)
            off += caps[k]
        xcs = np.ascontiguousarray(xc.T.reshape(ND, P, S).transpose(1, 0, 2))
        # w1/v1 f-unit swizzles, units slot-major
        w1z = np.concatenate(
            [_swizzle_w1(w1[groups[q][k]][rows].astype(np_bf16))
             for k in range(G)], axis=0)
        v1z = np.concatenate(
            [_swizzle_w1(v1[groups[q][k]][rows].astype(np_bf16))
             for k in range(G)], axis=0)
        wv = np.concatenate(
            [w1z[0::2], v1z[0::2], w1z[1::2], v1z[1::2]], axis=-1)
        wv = np.ascontiguousarray(wv)
        # w2 units: [p, u, d] = w2slice[u*128+p, d]
        w2z = np.concatenate(
            [w2[groups[q][k]][rows].astype(np_bf16).reshape(f_per, P, D)
             for k in range(G)], axis=0).transpose(1, 0, 2)
        w2z = np.ascontiguousarray(w2z)
        in_maps.append({"xcs": xcs, "wv1s": wv, "w2s": w2z})
    return in_maps


def _capacity(idx):
    groups, caps = _grouping(idx)
    return tuple(caps)


def kernel(x, gate_w, w1, v1, w2):
    idx, cf = _route(x, gate_w)
    groups, caps = _grouping(idx)
    nc = _get_nc(caps)
    in_maps = _build_in_maps(x, gate_w, w1, v1, w2, caps, idx)
    res = run_bass_kernel_spmd(nc, in_maps, core_ids=list(range(E)))
    out = np.zeros((T, D), dtype=np.float32)
    for q in range(E // G):
        ysum = sum(res.results[q * G + h]["yT"].astype(np.float32)
                   for h in range(G))
        off = 0
        for k in range(G):
            e = groups[q][k]
            n = len(idx[e])
            y = ysum.T[off:off + n]  # [n, D] unscaled expert output
            out[idx[e]] += cf[e][:, None] * y
            off += caps[k]
    return out
